# revision 11
# baseline (speedup 1.0000x reference)
# Trainium2 Bass kernel for nn_Adapter_22342419874228.
#
# Per row r of logits [B=16384, C=1000]:
#   prob = softmax(logits); order = argsort(-prob); sp = prob[order]
#   diffs = [sqrt(sp[j]-sp[j+1]) ... , 1]
#   raw = diffs * [sigmoid((prob@W.T+b)[:999]), (prob@W.T+b)[999]]
#   rc = reverse-cumsum(raw); fitted[r, order[j]] = rc[j]
#
# Device algorithm (per 128-row tile, data-parallel over 8 cores):
#   e = exp(logits) (no max-sub needed: logits in [-5.5, 5.5]); Z = rowsum(e)
#   sort key = (float32 bits of e with low 10 mantissa bits replaced by the
#   column index) -- a positive float whose fp32 min/max order equals the
#   (truncated-e, idx) lexicographic order, so a single-plane bitonic sort
#   carries values and indices together.  55-substage normalized bitonic
#   network (all compare-exchanges max-to-low) on the vector engine.
#   fc is computed as e_ext @ WT_ext where e_ext has an extra column = Z and
#   WT_ext an extra row = b, so fc/Z = prob@W.T + b without normalizing e.
#   sigmoid(x) = 0.5*(1+tanh(x/2)) keeps the scalar engine on one act table.
#   raw = (sqrt(gap)*0.5/sqrt(Z)) * (1+tanh(fc/(2Z))); rev-cumsum via scan;
#   fitted is produced by a per-partition local_scatter of rc at the sorted
#   indices.
import numpy as np
import ml_dtypes

import concourse.bass as bass
import concourse.tile as tile
from concourse import bacc
from concourse import mybir, library_config
from concourse.bass_utils import run_bass_kernel_spmd
from concourse.masks import make_identity

BATCH = 16384
C = 1000
NCORES = 8
ROWS = BATCH // NCORES        # 2048 rows per core
P = 128                       # partition tile
NTILES = ROWS // P            # 16
N = 1024                      # padded sort width
KEXT = 1008                   # contraction length: 1000 + Z col + 7 zero pad
KCH = 126                     # 8 chunks of 126 = 1008

AF = mybir.ActivationFunctionType
OP = mybir.AluOpType

# Tiles whose bitonic sort runs on the Pool (gpsimd) engine instead of DVE,
# to balance the two engines.  DVE sorts the rest.
POOL_SORT_TILES = frozenset()


def _int_bitop(eng, out, in0, imm, imm_dtype, op0, in1=None, op1=None):
    """Emit a TensorScalarPtr with an integer-typed immediate (the BIR
    verifier requires bitvec-op immediates to match the src/dst dtype)."""
    ins = [eng.lower_ap(in0), mybir.ImmediateValue(dtype=imm_dtype, value=imm)]
    kw = {}
    if in1 is not None:
        ins.append(eng.lower_ap(in1))
        kw = dict(is_scalar_tensor_tensor=True)
    return eng.add_instruction(mybir.InstTensorScalarPtr(
        name=eng.bass.get_next_instruction_name(),
        op0=op0, op1=(op1 if op1 is not None else OP.bypass),
        ins=ins, outs=[eng.lower_ap(out)], **kw))


def _ce_pair(eng, src, dst, lows_off, lows_dims, part_off, part_dims, use_tt):
    """One compare-exchange substage: max of (lows, partners) to the low
    positions of dst, min to the partner positions.  DVE uses the
    scalar_tensor_tensor form (0.5 cyc/elem with the 2x_2p perf mode); Pool
    only supports the plain tensor_tensor form in codegen."""
    mk = lambda t, off, dims: bass.AP(
        tensor=t.tensor, offset=t.offset + off, ap=[t.ap[0]] + dims
    )
    lows_src = mk(src, lows_off, lows_dims)
    part_src = mk(src, part_off, part_dims)
    lows_dst = mk(dst, lows_off, lows_dims)
    part_dst = mk(dst, part_off, part_dims)
    if use_tt:
        eng.tensor_tensor(out=lows_dst, in0=lows_src, in1=part_src, op=OP.max)
        eng.tensor_tensor(out=part_dst, in0=lows_src, in1=part_src, op=OP.min)
    else:
        eng.scalar_tensor_tensor(out=lows_dst, in0=lows_src, scalar=0.0,
                                 in1=part_src, op0=OP.bypass, op1=OP.max)
        eng.scalar_tensor_tensor(out=part_dst, in0=lows_src, scalar=0.0,
                                 in1=part_src, op0=OP.bypass, op1=OP.min)


def _emit_sort(eng, bufA, bufB, use_tt=False):
    """Normalized bitonic sort, descending, on packed positive-float keys.
    Returns the buffer holding the sorted result."""
    src, dst = bufA, bufB
    k = 2
    while k <= N:
        nb = N // k
        half = k // 2
        _ce_pair(eng, src, dst, 0, [[k, nb], [1, half]], k - 1,
                 [[k, nb], [-1, half]], use_tt)
        src, dst = dst, src
        j = k // 4
        while j >= 1:
            nb2 = N // (2 * j)
            _ce_pair(eng, src, dst, 0, [[2 * j, nb2], [1, j]], j,
                     [[2 * j, nb2], [1, j]], use_tt)
            src, dst = dst, src
            j //= 2
        k *= 2
    return src


def build_nc():
    nc = bacc.Bacc(None, target_bir_lowering=False)
    l_in = nc.dram_tensor("logits", [ROWS, C], mybir.dt.float32, kind="ExternalInput")
    wt_in = nc.dram_tensor("wt", [KEXT, C], mybir.dt.bfloat16, kind="ExternalInput")
    out_d = nc.dram_tensor("out", [ROWS, C], mybir.dt.float32, kind="ExternalOutput")

    with tile.TileContext(nc) as tc:
        with (
            tc.tile_pool(name="singles", bufs=1) as singles,
            tc.tile_pool(name="inp", bufs=3) as inp,
            tc.tile_pool(name="front", bufs=2) as front,
            tc.tile_pool(name="sortp", bufs=4) as sortp,
            tc.tile_pool(name="back", bufs=2) as back,
            tc.tile_pool(name="outp", bufs=3) as outp,
            tc.tile_pool(name="psum", bufs=2, space=bass.MemorySpace.PSUM) as psum,
        ):
            iota = singles.tile([P, N], mybir.dt.int32)
            ident = singles.tile([P, P], mybir.dt.float32)
            wt = singles.tile([KCH, 8, C], mybir.dt.bfloat16)

            nc.gpsimd.iota(iota[:], [[1, N]], channel_multiplier=0)
            make_identity(nc, ident[:])
            for k in range(8):
                nc.sync.dma_start(wt[:, k, :], wt_in[k * KCH:(k + 1) * KCH, :])
            nc.gpsimd.load_library(library_config.local_scatter)

            for t in range(NTILES):
                r0 = t * P
                lt = inp.tile([P, C], mybir.dt.float32)
                nc.sync.dma_start(lt[:], l_in[r0:r0 + P, :])

                e = front.tile([P, KEXT], mybir.dt.float32)
                Z = front.tile([P, 1], mybir.dt.float32)
                recipZ = front.tile([P, 1], mybir.dt.float32)
                halfRecipZ = front.tile([P, 1], mybir.dt.float32)
                qZ = front.tile([P, 1], mybir.dt.float32)
                t1 = front.tile([P, C], mybir.dt.float32)
                ebT = front.tile([P, 8, P], mybir.dt.bfloat16)

                # softmax numerator + partition function
                nc.scalar.activation(out=e[:, 0:C], in_=lt[:], func=AF.Exp,
                                     accum_out=Z[:])
                nc.scalar.activation(out=e[:, C:C + 1], in_=Z[:], func=AF.Copy)
                nc.vector.memset(e[:, C + 1:KEXT], 0.0)
                nc.vector.reciprocal(out=recipZ[:], in_=Z[:])
                nc.vector.tensor_scalar(out=halfRecipZ[:], in0=recipZ[:],
                                        scalar1=0.5, scalar2=None, op0=OP.mult)
                nc.vector.tensor_scalar(out=qZ[:], in0=recipZ[:],
                                        scalar1=0.25, scalar2=None, op0=OP.mult)

                # pack sort keys and sort
                bufA = sortp.tile([P, N], mybir.dt.float32)
                bufB = sortp.tile([P, N], mybir.dt.float32)
                _int_bitop(nc.vector, bufA[:, 0:C].bitcast(mybir.dt.int32),
                           e[:, 0:C].bitcast(mybir.dt.int32), -1024,
                           mybir.dt.int32, OP.bitwise_and,
                           in1=iota[:, 0:C], op1=OP.bitwise_or)
                nc.vector.memset(bufA[:, C:N], 0.0)
                on_pool = t in POOL_SORT_TILES
                spk = _emit_sort(nc.gpsimd if on_pool else nc.vector,
                                 bufA, bufB, use_tt=on_pool)

                # fc path: transpose e, matmul with WT_ext
                psT = psum.tile([P, 8, P], mybir.dt.float32)
                for k in range(8):
                    nc.tensor.transpose(psT[0:KCH, k, :], e[:, k * KCH:(k + 1) * KCH],
                                        ident[:])
                nc.scalar.activation(out=ebT[0:KCH], in_=psT[0:KCH], func=AF.Copy)
                psA = psum.tile([P, 512], mybir.dt.float32)
                psB = psum.tile([P, C - 512], mybir.dt.float32)
                for k in range(8):
                    nc.tensor.matmul(psA[:], ebT[0:KCH, k, :], wt[:, k, 0:512],
                                     start=(k == 0), stop=(k == 7))
                for k in range(8):
                    nc.tensor.matmul(psB[:], ebT[0:KCH, k, :], wt[:, k, 512:C],
                                     start=(k == 0), stop=(k == 7))

                # t1 = tanh(fc/(2Z)) (cols 0..998), t1[999] = fc[999]/Z - 1
                # (raw multiply below computes (t1+1)*ds, so col 999 carries
                # fc[999]-1 to make raw[999] = fc[999]/Z with ds[999] = 1)
                nc.scalar.activation(out=t1[:, 0:512], in_=psA[:], func=AF.Tanh,
                                     scale=halfRecipZ[:])
                nc.scalar.activation(out=t1[:, 512:C - 1], in_=psB[:, 0:C - 1 - 512],
                                     func=AF.Tanh, scale=halfRecipZ[:])
                nc.scalar.activation(out=t1[:, C - 1:C], in_=psB[:, C - 1 - 512:C - 512],
                                     func=AF.Copy, scale=recipZ[:], bias=-1.0)

                # unpack sorted keys -> truncated values + indices
                es = back.tile([P, C + 1], mybir.dt.float32)
                idx16 = back.tile([P, C], mybir.dt.int16)
                ds = back.tile([P, C], mybir.dt.float32)
                _int_bitop(nc.vector, es[:].bitcast(mybir.dt.int32),
                           spk[:, 0:C + 1].bitcast(mybir.dt.int32), -1024,
                           mybir.dt.int32, OP.bitwise_and)
                # low 16 bits of each packed key (even int16 elements) hold
                # idx in bits 0..9; mask in int16 directly.
                spk16 = spk[:].bitcast(mybir.dt.int16)
                spk16_even = bass.AP(tensor=spk16.tensor, offset=spk16.offset,
                                     ap=[spk16.ap[0], [2, C]])
                _int_bitop(nc.vector, idx16[:], spk16_even, 1023,
                           mybir.dt.int16, OP.bitwise_and)

                # diffs with normalization folded into the sqrt scale:
                # ds[j] = sqrt((es[j]-es[j+1]) * 0.25/Z) = 0.5*sqrt(gap)/sqrt(Z)
                nc.gpsimd.tensor_tensor(out=ds[:, 0:C - 1], in0=es[:, 0:C - 1],
                                        in1=es[:, 1:C], op=OP.subtract)
                nc.scalar.activation(out=ds[:, 0:C - 1], in_=ds[:, 0:C - 1],
                                     func=AF.Sqrt, scale=qZ[:])
                nc.vector.memset(ds[:, C - 1:C], 1.0)

                # raw = ds * (t1 + 1) ; T = rowsum(raw)
                raw = back.tile([P, C], mybir.dt.float32)
                T = back.tile([P, 1], mybir.dt.float32)
                Cs = back.tile([P, C + 1], mybir.dt.float32)
                rcb = back.tile([P, C], mybir.dt.bfloat16)
                nc.vector.scalar_tensor_tensor(out=raw[:], in0=t1[:],
                                               scalar=1.0, in1=ds[:],
                                               op0=OP.add, op1=OP.mult,
                                               accum_out=T[:])
                # reverse cumsum: rc[j] = T - cumsum(raw)[j-1]
                nc.gpsimd.memset(Cs[:, 0:1], 0.0)
                nc.vector.tensor_tensor_scan(out=Cs[:, 1:C + 1], data0=raw[:],
                                             data1=raw[:], initial=0.0,
                                             op0=OP.add, op1=OP.bypass)
                nc.vector.tensor_scalar(out=rcb[:], in0=Cs[:, 0:C], scalar1=T[:],
                                        scalar2=-1.0, op0=OP.subtract, op1=OP.mult)

                # scatter rc to class positions, widen to f32, store
                fitb = outp.tile([P, N], mybir.dt.bfloat16)
                fitf = outp.tile([P, C], mybir.dt.float32)
                nc.gpsimd.local_scatter(out_ap=fitb[:], data_ap=rcb[:],
                                        idxs_ap=idx16[:], channels=P,
                                        num_elems=N, num_idxs=C)
                nc.scalar.activation(out=fitf[:], in_=fitb[:, 0:C], func=AF.Copy)
                nc.sync.dma_start(out_d[r0:r0 + P, :], fitf[:])
    nc.compile()
    return nc


def _prep_wt(W, b):
    wt_ext = np.zeros((KEXT, C), dtype=ml_dtypes.bfloat16)
    wt_ext[:C, :] = W.T.astype(ml_dtypes.bfloat16)
    wt_ext[C, :] = b.astype(ml_dtypes.bfloat16)
    return wt_ext


def kernel(logits, W, b):
    logits = np.ascontiguousarray(np.asarray(logits, dtype=np.float32))
    W = np.asarray(W, dtype=np.float32)
    b = np.asarray(b, dtype=np.float32)
    assert logits.shape == (BATCH, C)
    wt_ext = _prep_wt(W, b)

    nc = build_nc()
    in_maps = [
        {"logits": logits[i * ROWS:(i + 1) * ROWS], "wt": wt_ext}
        for i in range(NCORES)
    ]
    res = run_bass_kernel_spmd(nc, in_maps, core_ids=list(range(NCORES)))
    out = np.concatenate([res.results[i]["out"] for i in range(NCORES)], axis=0)
    return out.astype(np.float32)


# revision 12
# speedup vs baseline: 1.0153x; 1.0153x over previous
# Trainium2 Bass kernel for nn_Adapter_22342419874228.
#
# Per row r of logits [B=16384, C=1000]:
#   prob = softmax(logits); order = argsort(-prob); sp = prob[order]
#   diffs = [sqrt(sp[j]-sp[j+1]) ... , 1]
#   raw = diffs * [sigmoid((prob@W.T+b)[:999]), (prob@W.T+b)[999]]
#   rc = reverse-cumsum(raw); fitted[r, order[j]] = rc[j]
#
# Device algorithm (per 128-row tile, data-parallel over 8 cores):
#   e = exp(logits) (no max-sub needed: logits in [-5.5, 5.5]); Z = rowsum(e)
#   sort key = (float32 bits of e with low 10 mantissa bits replaced by the
#   column index) -- a positive float whose fp32 min/max order equals the
#   (truncated-e, idx) lexicographic order, so a single-plane bitonic sort
#   carries values and indices together.  55-substage normalized bitonic
#   network (all compare-exchanges max-to-low) on the vector engine.
#   fc is computed as e_ext @ WT_ext where e_ext has an extra column = Z and
#   WT_ext an extra row = b, so fc/Z = prob@W.T + b without normalizing e.
#   sigmoid(x) = 0.5*(1+tanh(x/2)) keeps the scalar engine on one act table.
#   raw = (sqrt(gap)*0.5/sqrt(Z)) * (1+tanh(fc/(2Z))); rev-cumsum via scan;
#   fitted is produced by a per-partition local_scatter of rc at the sorted
#   indices.
import numpy as np
import ml_dtypes

import concourse.bass as bass
import concourse.tile as tile
from concourse import bacc
from concourse import mybir, library_config
from concourse.bass_utils import run_bass_kernel_spmd
from concourse.masks import make_identity

BATCH = 16384
C = 1000
NCORES = 8
ROWS = BATCH // NCORES        # 2048 rows per core
P = 128                       # partition tile
NTILES = ROWS // P            # 16
N = 1024                      # padded sort width
KEXT = 1008                   # contraction length: 1000 + Z col + 7 zero pad
KCH = 126                     # 8 chunks of 126 = 1008

AF = mybir.ActivationFunctionType
OP = mybir.AluOpType

def _int_bitop(eng, out, in0, imm, imm_dtype, op0, in1=None, op1=None):
    """Emit a TensorScalarPtr with an integer-typed immediate (the BIR
    verifier requires bitvec-op immediates to match the src/dst dtype)."""
    ins = [eng.lower_ap(in0), mybir.ImmediateValue(dtype=imm_dtype, value=imm)]
    kw = {}
    if in1 is not None:
        ins.append(eng.lower_ap(in1))
        kw = dict(is_scalar_tensor_tensor=True)
    return eng.add_instruction(mybir.InstTensorScalarPtr(
        name=eng.bass.get_next_instruction_name(),
        op0=op0, op1=(op1 if op1 is not None else OP.bypass),
        ins=ins, outs=[eng.lower_ap(out)], **kw))


def _ce_pair(eng, src, dst, lows_off, lows_dims, part_off, part_dims):
    """One compare-exchange substage: max of (lows, partners) to the low
    positions of dst, min to the partner positions.  Plain tensor_tensor is
    the fastest DVE form on hardware (414ns vs 681ns for
    scalar_tensor_tensor at 512 free elems)."""
    mk = lambda t, off, dims: bass.AP(
        tensor=t.tensor, offset=t.offset + off, ap=[t.ap[0]] + dims
    )
    lows_src = mk(src, lows_off, lows_dims)
    part_src = mk(src, part_off, part_dims)
    lows_dst = mk(dst, lows_off, lows_dims)
    part_dst = mk(dst, part_off, part_dims)
    eng.tensor_tensor(out=lows_dst, in0=lows_src, in1=part_src, op=OP.max)
    eng.tensor_tensor(out=part_dst, in0=lows_src, in1=part_src, op=OP.min)


def _emit_sort(eng, bufA, bufB):
    """Normalized bitonic sort, descending, on packed positive-float keys.
    Returns the buffer holding the sorted result."""
    src, dst = bufA, bufB
    k = 2
    while k <= N:
        nb = N // k
        half = k // 2
        _ce_pair(eng, src, dst, 0, [[k, nb], [1, half]], k - 1,
                 [[k, nb], [-1, half]])
        src, dst = dst, src
        j = k // 4
        while j >= 1:
            nb2 = N // (2 * j)
            _ce_pair(eng, src, dst, 0, [[2 * j, nb2], [1, j]], j,
                     [[2 * j, nb2], [1, j]])
            src, dst = dst, src
            j //= 2
        k *= 2
    return src


def build_nc():
    nc = bacc.Bacc(None, target_bir_lowering=False)
    l_in = nc.dram_tensor("logits", [ROWS, C], mybir.dt.float32, kind="ExternalInput")
    wt_in = nc.dram_tensor("wt", [KEXT, C], mybir.dt.bfloat16, kind="ExternalInput")
    out_d = nc.dram_tensor("out", [ROWS, C], mybir.dt.float32, kind="ExternalOutput")

    with tile.TileContext(nc) as tc:
        with (
            tc.tile_pool(name="singles", bufs=1) as singles,
            tc.tile_pool(name="inp", bufs=3) as inp,
            tc.tile_pool(name="front", bufs=2) as front,
            tc.tile_pool(name="sortp", bufs=4) as sortp,
            tc.tile_pool(name="back", bufs=2) as back,
            tc.tile_pool(name="outp", bufs=3) as outp,
            tc.tile_pool(name="psum", bufs=2, space=bass.MemorySpace.PSUM) as psum,
        ):
            iota = singles.tile([P, N], mybir.dt.int32)
            ident = singles.tile([P, P], mybir.dt.float32)
            wt = singles.tile([KCH, 8, C], mybir.dt.bfloat16)
            ones = singles.tile([P, C], mybir.dt.float32)
            m1024 = singles.tile([P, C + 1], mybir.dt.int32)

            nc.gpsimd.iota(iota[:], [[1, N]], channel_multiplier=0)
            make_identity(nc, ident[:])
            nc.vector.memset(ones[:], 1.0)
            nc.vector.memset(m1024[:], -1024)
            for k in range(8):
                nc.sync.dma_start(wt[:, k, :], wt_in[k * KCH:(k + 1) * KCH, :])
            nc.gpsimd.load_library(library_config.local_scatter)

            for t in range(NTILES):
                r0 = t * P
                lt = inp.tile([P, C], mybir.dt.float32)
                nc.sync.dma_start(lt[:], l_in[r0:r0 + P, :])

                e = front.tile([P, KEXT], mybir.dt.float32)
                Z = front.tile([P, 1], mybir.dt.float32)
                recipZ = front.tile([P, 1], mybir.dt.float32)
                halfRecipZ = front.tile([P, 1], mybir.dt.float32)
                qZ = front.tile([P, 1], mybir.dt.float32)
                t1 = front.tile([P, C], mybir.dt.float32)
                ebT = front.tile([P, 8, P], mybir.dt.bfloat16)

                # softmax numerator + partition function
                nc.scalar.activation(out=e[:, 0:C], in_=lt[:], func=AF.Exp,
                                     accum_out=Z[:])
                nc.scalar.activation(out=e[:, C:C + 1], in_=Z[:], func=AF.Copy)
                nc.scalar.memzero(e[:, C + 1:KEXT])
                nc.vector.reciprocal(out=recipZ[:], in_=Z[:])
                nc.vector.tensor_scalar(out=halfRecipZ[:], in0=recipZ[:],
                                        scalar1=0.5, scalar2=None, op0=OP.mult)
                nc.vector.tensor_scalar(out=qZ[:], in0=recipZ[:],
                                        scalar1=0.25, scalar2=None, op0=OP.mult)

                # pack sort keys and sort
                bufA = sortp.tile([P, N], mybir.dt.float32)
                bufB = sortp.tile([P, N], mybir.dt.float32)
                bufAi = bufA[:, 0:C].bitcast(mybir.dt.int32)
                nc.vector.tensor_tensor(out=bufAi, in0=e[:, 0:C].bitcast(mybir.dt.int32),
                                        in1=m1024[:, 0:C], op=OP.bitwise_and)
                nc.vector.tensor_tensor(out=bufAi, in0=bufAi, in1=iota[:, 0:C],
                                        op=OP.bitwise_or)
                nc.vector.memset(bufA[:, C:N], 0.0)
                spk = _emit_sort(nc.vector, bufA, bufB)

                # fc path: transpose e, matmul with WT_ext
                psT = psum.tile([P, 8, P], mybir.dt.float32)
                for k in range(8):
                    nc.tensor.transpose(psT[0:KCH, k, :], e[:, k * KCH:(k + 1) * KCH],
                                        ident[:])
                nc.scalar.activation(out=ebT[0:KCH], in_=psT[0:KCH], func=AF.Copy)
                psA = psum.tile([P, 512], mybir.dt.float32)
                psB = psum.tile([P, C - 512], mybir.dt.float32)
                for k in range(8):
                    nc.tensor.matmul(psA[:], ebT[0:KCH, k, :], wt[:, k, 0:512],
                                     start=(k == 0), stop=(k == 7))
                for k in range(8):
                    nc.tensor.matmul(psB[:], ebT[0:KCH, k, :], wt[:, k, 512:C],
                                     start=(k == 0), stop=(k == 7))

                # t1 = tanh(fc/(2Z)) (cols 0..998), t1[999] = fc[999]/Z - 1
                # (raw multiply below computes (t1+1)*ds, so col 999 carries
                # fc[999]-1 to make raw[999] = fc[999]/Z with ds[999] = 1)
                nc.scalar.activation(out=t1[:, 0:512], in_=psA[:], func=AF.Tanh,
                                     scale=halfRecipZ[:])
                nc.scalar.activation(out=t1[:, 512:C - 1], in_=psB[:, 0:C - 1 - 512],
                                     func=AF.Tanh, scale=halfRecipZ[:])
                nc.scalar.activation(out=t1[:, C - 1:C], in_=psB[:, C - 1 - 512:C - 512],
                                     func=AF.Copy, scale=recipZ[:], bias=-1.0)

                # unpack sorted keys -> truncated values + indices
                es = back.tile([P, C + 1], mybir.dt.float32)
                idx16 = back.tile([P, C], mybir.dt.int16)
                ds = back.tile([P, C], mybir.dt.float32)
                nc.vector.tensor_tensor(out=es[:].bitcast(mybir.dt.int32),
                                        in0=spk[:, 0:C + 1].bitcast(mybir.dt.int32),
                                        in1=m1024[:], op=OP.bitwise_and)
                # low 16 bits of each packed key (even int16 elements) hold
                # idx in bits 0..9; mask in int16 directly.
                spk16 = spk[:].bitcast(mybir.dt.int16)
                spk16_even = bass.AP(tensor=spk16.tensor, offset=spk16.offset,
                                     ap=[spk16.ap[0], [2, C]])
                _int_bitop(nc.vector, idx16[:], spk16_even, 1023,
                           mybir.dt.int16, OP.bitwise_and)

                # diffs with normalization folded into the sqrt scale:
                # ds[j] = sqrt((es[j]-es[j+1]) * 0.25/Z) = 0.5*sqrt(gap)/sqrt(Z)
                nc.gpsimd.tensor_tensor(out=ds[:, 0:C - 1], in0=es[:, 0:C - 1],
                                        in1=es[:, 1:C], op=OP.subtract)
                nc.scalar.activation(out=ds[:, 0:C - 1], in_=ds[:, 0:C - 1],
                                     func=AF.Sqrt, scale=qZ[:])
                nc.vector.memset(ds[:, C - 1:C], 1.0)

                # raw = ds * (t1 + 1): in-place add then multiply
                raw = back.tile([P, C], mybir.dt.float32)
                Cs = back.tile([P, C + 1], mybir.dt.float32)
                rcb = back.tile([P, C], mybir.dt.bfloat16)
                nc.vector.tensor_tensor(out=t1[:], in0=t1[:], in1=ones[:], op=OP.add)
                nc.vector.tensor_tensor(out=raw[:], in0=t1[:], in1=ds[:], op=OP.mult)
                # inclusive cumsum; its tail Cs[:,C] is the row total T
                nc.gpsimd.memset(Cs[:, 0:1], 0.0)
                nc.vector.tensor_tensor_scan(out=Cs[:, 1:C + 1], data0=raw[:],
                                             data1=raw[:], initial=0.0,
                                             op0=OP.add, op1=OP.bypass)
                # rc[j] = T - Cs[j] = Identity(Cs[j]*-1 + T), cast to bf16
                nc.scalar.activation(out=rcb[:], in_=Cs[:, 0:C], func=AF.Identity,
                                     scale=-1.0, bias=Cs[:, C:C + 1])

                # scatter rc to class positions, widen to f32, store
                fitb = outp.tile([P, N], mybir.dt.bfloat16)
                fitf = outp.tile([P, C], mybir.dt.float32)
                nc.gpsimd.local_scatter(out_ap=fitb[:], data_ap=rcb[:],
                                        idxs_ap=idx16[:], channels=P,
                                        num_elems=N, num_idxs=C)
                nc.scalar.activation(out=fitf[:], in_=fitb[:, 0:C], func=AF.Copy)
                nc.sync.dma_start(out_d[r0:r0 + P, :], fitf[:])
    nc.compile()
    return nc


def _prep_wt(W, b):
    wt_ext = np.zeros((KEXT, C), dtype=ml_dtypes.bfloat16)
    wt_ext[:C, :] = W.T.astype(ml_dtypes.bfloat16)
    wt_ext[C, :] = b.astype(ml_dtypes.bfloat16)
    return wt_ext


def kernel(logits, W, b):
    logits = np.ascontiguousarray(np.asarray(logits, dtype=np.float32))
    W = np.asarray(W, dtype=np.float32)
    b = np.asarray(b, dtype=np.float32)
    assert logits.shape == (BATCH, C)
    wt_ext = _prep_wt(W, b)

    nc = build_nc()
    in_maps = [
        {"logits": logits[i * ROWS:(i + 1) * ROWS], "wt": wt_ext}
        for i in range(NCORES)
    ]
    res = run_bass_kernel_spmd(nc, in_maps, core_ids=list(range(NCORES)))
    out = np.concatenate([res.results[i]["out"] for i in range(NCORES)], axis=0)
    return out.astype(np.float32)
